# revision 8
# baseline (speedup 1.0000x reference)
"""Trainium2 Bass kernel for 3x3 VALID conv: x[32,128,64,64] * w[256,128,3,3] + bias.

Strategy:
  - Data-parallel over batch: 8 cores x 4 images each; weights/bias replicated.
  - Per core: implicit GEMM in bf16. Contraction dim = C_IN = 128 = partition
    dim. For each filter tap (u,v), accumulate into PSUM
        psum[o, i, j] += W[c, o; u,v].T @ x[c, (i+u)*64 + j + v]
    over chunks of 8 output rows x 62 valid cols (moving operand is a 2D
    access pattern [(rows, stride 64), (62, stride 1)] - no wasted columns).
  - bf16 matmuls stream ~1.03 cycles/row on the PE (vs 1.11 for fp32r);
    rel err ~1e-3, far below the 2e-2 gate.
  - PE warmup: a few matmuls on a scratch tile raise the PE p-state while
    the first input tiles are still in flight on the DMA rings.
  - DMA rings: x on the GpSimd ring, w+bias on the Scalar ring, outputs on
    the Sync ring - triggers never queue behind each other across tensors.
  - PSUM evacuation + bias add alternates between VectorE (tensor_scalar)
    and ScalarE (activation identity w/ per-partition bias) per half.
"""

import numpy as np
import ml_dtypes

import concourse.bacc as bacc
import concourse.tile as tile
from concourse import mybir
from concourse.ap import AP
from concourse.bass_utils import run_bass_kernel_spmd

N_CORES = 8
B_FULL, C_IN, H, W = 32, 128, 64, 64
C_OUT, KH, KW = 256, 3, 3
B_LOC = B_FULL // N_CORES          # images per core
H_OUT, W_OUT = H - KH + 1, W - KW + 1   # 62, 62
N_HALF = C_OUT // 128              # 2 output-channel halves
ROWS_PER_CHUNK = 8                 # 8 out rows x 62 cols = 496 <= one PSUM bank
N_CHUNKS = (H_OUT + ROWS_PER_CHUNK - 1) // ROWS_PER_CHUNK
# x piece boundaries in flattened (h w) cols: chunk c needs rows <= 8c+9.
X_PIECES = [(0, 640), (640, 2048), (2048, H * W)]
WARMUP_MMS = 6

_cached = {}


def _build_nc():
    f32 = mybir.dt.float32
    bf16 = mybir.dt.bfloat16
    nc = bacc.Bacc()

    x_d = nc.declare_dram_parameter("x", [B_LOC, C_IN, H * W], bf16, isOutput=False)
    w_d = nc.declare_dram_parameter(
        "w", [C_IN, N_HALF, KH * KW, 128], bf16, isOutput=False
    )
    b_d = nc.declare_dram_parameter("bias_in", [128, N_HALF], f32, isOutput=False)
    y_d = nc.declare_dram_parameter(
        "y", [B_LOC, N_HALF, 128, H_OUT, W_OUT], f32, isOutput=True
    )

    with tile.TileContext(nc) as tc:
        with (
            tc.tile_pool(name="const", bufs=1) as cpool,
            tc.tile_pool(name="xin", bufs=2) as xpool,
            tc.tile_pool(name="out", bufs=4) as opool,
            tc.tile_pool(name="psum", bufs=6, space="PSUM") as ppool,
            tc.tile_pool(name="warmps", bufs=2, space="PSUM") as wpool,
        ):
            w_t = cpool.tile([C_IN, N_HALF, KH * KW, 128], bf16)
            b_t = cpool.tile([128, N_HALF], f32)
            scratch = cpool.tile([128, 624], bf16)

            # Scalar ring: chunk0's half0 taps first (critical path), then the
            # rest of the weights, then bias.
            nc.scalar.dma_start(w_t[:, 0, 0:3], w_d[:, 0, 0:3])
            nc.scalar.dma_start(w_t[:, 1, 0:3], w_d[:, 1, 0:3])
            nc.scalar.dma_start(w_t[:, :, 3 : KH * KW], w_d[:, :, 3 : KH * KW])
            nc.scalar.dma_start(b_t[:], b_d[:])

            # PE warmup on scratch (raises p-state while DMA lands).
            nc.vector.memset(scratch[:], 0)
            for i in range(WARMUP_MMS):
                wps = wpool.tile([128, 496], f32, tag="warm")
                nc.tensor.matmul(
                    wps[:, 0:248],
                    scratch[:, 0:128],
                    scratch[:, 128:376],
                    start=True,
                    stop=True,
                )

            def load_x(b):
                x_t = xpool.tile([C_IN, H * W], bf16, tag="x")
                if b == 0:
                    # Stripe chunk0's rows across two idle rings so the
                    # first matmul data lands earlier.
                    nc.gpsimd.dma_start(x_t[:, 0:320], x_d[b, :, 0:320])
                    nc.sync.dma_start(x_t[:, 320:640], x_d[b, :, 320:640])
                    nc.gpsimd.dma_start(x_t[:, 640:2048], x_d[b, :, 640:2048])
                    nc.sync.dma_start(x_t[:, 2048:4096], x_d[b, :, 2048:4096])
                else:
                    nc.gpsimd.dma_start(x_t[:, 0:2048], x_d[b, :, 0:2048])
                    nc.gpsimd.dma_start(x_t[:, 2048:4096], x_d[b, :, 2048:4096])
                return x_t

            def moving_ap(x_t, shift, r):
                a = x_t[:, shift : shift + 1]
                return AP(
                    a.tensor, a.offset,
                    [[int(a.ap[0][0]), 128], [W, r], [1, W_OUT]],
                )

            for b in range(B_LOC):
                x_t = load_x(b)
                chunks = [
                    (c * ROWS_PER_CHUNK, min(ROWS_PER_CHUNK, H_OUT - c * ROWS_PER_CHUNK))
                    for c in range(N_CHUNKS)
                ]
                if b == B_LOC - 1:
                    # Split the final chunk so the last output DMA overlaps
                    # the tail of the matmul stream.
                    i_last, r_last = chunks.pop()
                    chunks += [(i_last, r_last - 3), (i_last + r_last - 3, 3)]
                for i0, r in chunks:
                    for half in range(N_HALF):
                        ps = ppool.tile([128, ROWS_PER_CHUNK, W_OUT], f32, tag="ps")
                        for uv in range(KH * KW):
                            u, v = divmod(uv, KW)
                            shift = (i0 + u) * W + v
                            nc.tensor.matmul(
                                ps[:, 0:r, :],
                                w_t[:, half, uv, :],
                                moving_ap(x_t, shift, r),
                                start=(uv == 0),
                                stop=(uv == KH * KW - 1),
                            )
                        o_t = opool.tile([128, ROWS_PER_CHUNK, W_OUT], f32, tag="o")
                        if half == 0:
                            nc.vector.tensor_scalar_add(
                                o_t[:, 0:r, :], ps[:, 0:r, :], b_t[:, 0:1]
                            )
                        else:
                            nc.scalar.activation(
                                o_t[:, 0:r, :],
                                ps[:, 0:r, :],
                                mybir.ActivationFunctionType.Identity,
                                bias=b_t[:, 1:2],
                            )
                        nc.sync.dma_start(
                            y_d[b, half, :, i0 : i0 + r, :], o_t[:, 0:r, :]
                        )

    nc.compile()
    if not nc.is_finalized():
        nc.finalize()
    return nc


def kernel(inputs, weights, bias, profile=False, trace_kwargs=None):
    x_bf = np.ascontiguousarray(
        np.asarray(inputs, dtype=np.float32).reshape(B_FULL, C_IN, H * W)
    ).astype(ml_dtypes.bfloat16)
    # [O, C, KH, KW] -> [C, half, KH*KW, o_local]  (lhsT layout)
    w_bf = np.ascontiguousarray(
        np.asarray(weights, dtype=np.float32)
        .reshape(N_HALF, 128, C_IN, KH * KW)
        .transpose(2, 0, 3, 1)
    ).astype(ml_dtypes.bfloat16)
    b_t = np.ascontiguousarray(
        np.asarray(bias, dtype=np.float32).reshape(N_HALF, 128).T
    )

    if "nc" not in _cached:
        _cached["nc"] = _build_nc()
    nc = _cached["nc"]

    in_maps = [
        {
            "x": x_bf[i * B_LOC : (i + 1) * B_LOC],
            "w": w_bf,
            "bias_in": b_t,
        }
        for i in range(N_CORES)
    ]
    res = run_bass_kernel_spmd(
        nc,
        in_maps,
        list(range(N_CORES)),
        trace=profile,
        **(trace_kwargs or {}),
    )
    _cached["last_result"] = res

    shards = []
    for i in range(N_CORES):
        y = res.results[i]["y"]  # [B_LOC, 2, 128, 62, 62]
        shards.append(y.reshape(B_LOC, C_OUT, H_OUT, W_OUT))
    return np.ascontiguousarray(np.concatenate(shards, axis=0), dtype=np.float32)


# revision 9
# speedup vs baseline: 1.1634x; 1.1634x over previous
"""Trainium2 Bass kernel for 3x3 VALID conv: x[32,128,64,64] * w[256,128,3,3] + bias.

Strategy:
  - Data-parallel over batch: 8 cores x 4 images each; weights/bias replicated.
  - Per core: implicit GEMM in bf16. Contraction dim = C_IN = 128 = partition
    dim. For each filter tap (u,v), accumulate into PSUM
        psum[o, i, j] += W[c, o; u,v].T @ x[c, (i+u)*64 + j + v]
    over chunks of 8 output rows x 62 valid cols (moving operand is a 2D
    access pattern [(rows, stride 64), (62, stride 1)] - no wasted columns).
  - bf16 matmuls stream ~1.03 cycles/row on the PE (vs 1.11 for fp32r);
    rel err ~1e-3, far below the 2e-2 gate.
  - PE warmup: a few matmuls on a scratch tile raise the PE p-state while
    the first input tiles are still in flight on the DMA rings.
  - DMA rings: x on the GpSimd ring, w+bias on the Scalar ring, outputs on
    the Sync ring - triggers never queue behind each other across tensors.
  - PSUM evacuation + bias add alternates between VectorE (tensor_scalar)
    and ScalarE (activation identity w/ per-partition bias) per half.
"""

import numpy as np
import ml_dtypes

import concourse.bacc as bacc
import concourse.tile as tile
from concourse import mybir
from concourse.ap import AP
from concourse.bass_utils import run_bass_kernel_spmd

N_CORES = 8
B_FULL, C_IN, H, W = 32, 128, 64, 64
C_OUT, KH, KW = 256, 3, 3
B_LOC = B_FULL // N_CORES          # images per core
H_OUT, W_OUT = H - KH + 1, W - KW + 1   # 62, 62
N_HALF = C_OUT // 128              # 2 output-channel halves
ROWS_PER_CHUNK = 8                 # 8 out rows x 62 cols = 496 <= one PSUM bank
N_CHUNKS = (H_OUT + ROWS_PER_CHUNK - 1) // ROWS_PER_CHUNK
# x piece boundaries in flattened (h w) cols: chunk c needs rows <= 8c+9.
X_PIECES = [(0, 640), (640, 2048), (2048, H * W)]
WARMUP_MMS = 8

_cached = {}


def _build_nc():
    f32 = mybir.dt.float32
    bf16 = mybir.dt.bfloat16
    nc = bacc.Bacc()

    x_d = nc.declare_dram_parameter("x", [B_LOC, C_IN, H * W], bf16, isOutput=False)
    w_d = nc.declare_dram_parameter(
        "w", [C_IN, N_HALF, KH * KW, 128], bf16, isOutput=False
    )
    b_d = nc.declare_dram_parameter("bias_in", [128, N_HALF], f32, isOutput=False)
    y_d = nc.declare_dram_parameter(
        "y", [B_LOC, N_HALF, 128, H_OUT, W_OUT], f32, isOutput=True
    )

    with tile.TileContext(nc) as tc:
        with (
            tc.tile_pool(name="const", bufs=1) as cpool,
            tc.tile_pool(name="xin", bufs=2) as xpool,
            tc.tile_pool(name="out", bufs=4) as opool,
            tc.tile_pool(name="psum", bufs=6, space="PSUM") as ppool,
            tc.tile_pool(name="warmps", bufs=2, space="PSUM") as wpool,
        ):
            w_t = cpool.tile([C_IN, N_HALF, KH * KW, 128], bf16)
            b_t = cpool.tile([128, N_HALF], f32)
            scratch = cpool.tile([128, 624], bf16)

            # Scalar ring: chunk0's half0 taps first (critical path), then the
            # rest of the weights, then bias.
            nc.scalar.dma_start(w_t[:, 0, 0:3], w_d[:, 0, 0:3])
            nc.scalar.dma_start(w_t[:, 1, 0:3], w_d[:, 1, 0:3])
            nc.scalar.dma_start(w_t[:, :, 3 : KH * KW], w_d[:, :, 3 : KH * KW])
            nc.scalar.dma_start(b_t[:], b_d[:])

            # PE warmup on scratch (raises p-state while DMA lands).
            nc.vector.memset(scratch[:], 0)
            for i in range(WARMUP_MMS):
                wps = wpool.tile([128, 496], f32, tag="warm")
                nc.tensor.matmul(
                    wps[:, 0:248],
                    scratch[:, 0:128],
                    scratch[:, 128:376],
                    start=True,
                    stop=True,
                )

            def load_x(b):
                x_t = xpool.tile([C_IN, H * W], bf16, tag="x")
                if b == 0:
                    # Stripe chunk0's rows across two idle rings so the
                    # first matmul data lands earlier.
                    nc.gpsimd.dma_start(x_t[:, 0:320], x_d[b, :, 0:320])
                    nc.sync.dma_start(x_t[:, 320:640], x_d[b, :, 320:640])
                    nc.gpsimd.dma_start(x_t[:, 640:2048], x_d[b, :, 640:2048])
                    nc.sync.dma_start(x_t[:, 2048:4096], x_d[b, :, 2048:4096])
                else:
                    nc.gpsimd.dma_start(x_t[:, 0:2048], x_d[b, :, 0:2048])
                    nc.gpsimd.dma_start(x_t[:, 2048:4096], x_d[b, :, 2048:4096])
                return x_t

            def moving_ap(x_t, shift, r):
                a = x_t[:, shift : shift + 1]
                return AP(
                    a.tensor, a.offset,
                    [[int(a.ap[0][0]), 128], [W, r], [1, W_OUT]],
                )

            for b in range(B_LOC):
                x_t = load_x(b)
                chunks = [
                    (c * ROWS_PER_CHUNK, min(ROWS_PER_CHUNK, H_OUT - c * ROWS_PER_CHUNK))
                    for c in range(N_CHUNKS)
                ]
                if b == B_LOC - 1:
                    # Split the final chunk so the last output DMA overlaps
                    # the tail of the matmul stream.
                    i_last, r_last = chunks.pop()
                    chunks += [(i_last, r_last - 3), (i_last + r_last - 3, 3)]
                for i0, r in chunks:
                    for half in range(N_HALF):
                        ps = ppool.tile([128, ROWS_PER_CHUNK, W_OUT], f32, tag="ps")
                        for uv in range(KH * KW):
                            u, v = divmod(uv, KW)
                            shift = (i0 + u) * W + v
                            nc.tensor.matmul(
                                ps[:, 0:r, :],
                                w_t[:, half, uv, :],
                                moving_ap(x_t, shift, r),
                                start=(uv == 0),
                                stop=(uv == KH * KW - 1),
                            )
                        o_t = opool.tile([128, ROWS_PER_CHUNK, W_OUT], f32, tag="o")
                        if half == 0:
                            nc.vector.tensor_scalar_add(
                                o_t[:, 0:r, :], ps[:, 0:r, :], b_t[:, 0:1]
                            )
                        else:
                            nc.scalar.activation(
                                o_t[:, 0:r, :],
                                ps[:, 0:r, :],
                                mybir.ActivationFunctionType.Identity,
                                bias=b_t[:, 1:2],
                            )
                        nc.sync.dma_start(
                            y_d[b, half, :, i0 : i0 + r, :], o_t[:, 0:r, :]
                        )

    nc.compile()
    if not nc.is_finalized():
        nc.finalize()
    return nc


def kernel(inputs, weights, bias, profile=False, trace_kwargs=None):
    x_bf = np.ascontiguousarray(
        np.asarray(inputs, dtype=np.float32).reshape(B_FULL, C_IN, H * W)
    ).astype(ml_dtypes.bfloat16)
    # [O, C, KH, KW] -> [C, half, KH*KW, o_local]  (lhsT layout)
    w_bf = np.ascontiguousarray(
        np.asarray(weights, dtype=np.float32)
        .reshape(N_HALF, 128, C_IN, KH * KW)
        .transpose(2, 0, 3, 1)
    ).astype(ml_dtypes.bfloat16)
    b_t = np.ascontiguousarray(
        np.asarray(bias, dtype=np.float32).reshape(N_HALF, 128).T
    )

    if "nc" not in _cached:
        _cached["nc"] = _build_nc()
    nc = _cached["nc"]

    in_maps = [
        {
            "x": x_bf[i * B_LOC : (i + 1) * B_LOC],
            "w": w_bf,
            "bias_in": b_t,
        }
        for i in range(N_CORES)
    ]
    res = run_bass_kernel_spmd(
        nc,
        in_maps,
        list(range(N_CORES)),
        trace=profile,
        **(trace_kwargs or {}),
    )
    _cached["last_result"] = res

    shards = []
    for i in range(N_CORES):
        y = res.results[i]["y"]  # [B_LOC, 2, 128, 62, 62]
        shards.append(y.reshape(B_LOC, C_OUT, H_OUT, W_OUT))
    return np.ascontiguousarray(np.concatenate(shards, axis=0), dtype=np.float32)


# revision 11
# speedup vs baseline: 1.1870x; 1.0203x over previous
"""Trainium2 Bass kernel for 3x3 VALID conv: x[32,128,64,64] * w[256,128,3,3] + bias.

Strategy:
  - Data-parallel over batch: 8 cores x 4 images each; weights/bias replicated.
  - Per core: implicit GEMM in bf16. Contraction dim = C_IN = 128 = partition
    dim. For each filter tap (u,v), accumulate into PSUM
        psum[o, i, j] += W[c, o; u,v].T @ x[c, (i+u)*64 + j + v]
    over chunks of 8 output rows x 62 valid cols (moving operand is a 2D
    access pattern [(rows, stride 64), (62, stride 1)] - no wasted columns).
  - bf16 matmuls stream ~1.03 cycles/row on the PE (vs 1.11 for fp32r);
    rel err ~1e-3, far below the 2e-2 gate.
  - PE warmup: a few matmuls on a scratch tile raise the PE p-state while
    the first input tiles are still in flight on the DMA rings.
  - DMA rings: x on the GpSimd ring, w+bias on the Scalar ring, outputs on
    the Sync ring - triggers never queue behind each other across tensors.
  - PSUM evacuation + bias add alternates between VectorE (tensor_scalar)
    and ScalarE (activation identity w/ per-partition bias) per half.
"""

import numpy as np
import ml_dtypes

import concourse.bacc as bacc
import concourse.tile as tile
from concourse import mybir
from concourse.ap import AP
from concourse.bass_utils import run_bass_kernel_spmd

N_CORES = 8
B_FULL, C_IN, H, W = 32, 128, 64, 64
C_OUT, KH, KW = 256, 3, 3
B_LOC = B_FULL // N_CORES          # images per core
H_OUT, W_OUT = H - KH + 1, W - KW + 1   # 62, 62
N_HALF = C_OUT // 128              # 2 output-channel halves
ROWS_PER_CHUNK = 8                 # 8 out rows x 62 cols = 496 <= one PSUM bank
N_CHUNKS = (H_OUT + ROWS_PER_CHUNK - 1) // ROWS_PER_CHUNK
# x piece boundaries in flattened (h w) cols: chunk c needs rows <= 8c+9.
X_PIECES = [(0, 640), (640, 2048), (2048, H * W)]
WARMUP_MMS = 5

_cached = {}


def _build_nc():
    f32 = mybir.dt.float32
    bf16 = mybir.dt.bfloat16
    nc = bacc.Bacc()

    x_d = nc.declare_dram_parameter("x", [B_LOC, C_IN, H * W], bf16, isOutput=False)
    w_d = nc.declare_dram_parameter(
        "w", [C_IN, N_HALF, KH * KW, 128], bf16, isOutput=False
    )
    b_d = nc.declare_dram_parameter("bias_in", [128, N_HALF], f32, isOutput=False)
    y_d = nc.declare_dram_parameter(
        "y", [B_LOC, N_HALF, 128, H_OUT, W_OUT], f32, isOutput=True
    )

    with tile.TileContext(nc) as tc:
        with (
            tc.tile_pool(name="const", bufs=1) as cpool,
            tc.tile_pool(name="xin", bufs=2) as xpool,
            tc.tile_pool(name="out", bufs=4) as opool,
            tc.tile_pool(name="psum", bufs=6, space="PSUM") as ppool,
            tc.tile_pool(name="warmps", bufs=2, space="PSUM") as wpool,
        ):
            w_t = cpool.tile([C_IN, N_HALF, KH * KW, 128], bf16)
            b_t = cpool.tile([128, N_HALF], f32)
            scratch = cpool.tile([128, 624], bf16)

            # Scalar ring: chunk0's half0 taps first (critical path), then the
            # rest of the weights, then bias.
            nc.scalar.dma_start(w_t[:, 0, 0:3], w_d[:, 0, 0:3])
            nc.scalar.dma_start(w_t[:, 1, 0:3], w_d[:, 1, 0:3])
            nc.scalar.dma_start(w_t[:, :, 3 : KH * KW], w_d[:, :, 3 : KH * KW])
            nc.scalar.dma_start(b_t[:], b_d[:])

            # PE warmup on scratch (raises p-state while DMA lands).
            nc.vector.memset(scratch[:], 0)
            for i in range(WARMUP_MMS):
                wps = wpool.tile([128, 496], f32, tag="warm")
                nc.tensor.matmul(
                    wps[:, 0:248],
                    scratch[:, 0:128],
                    scratch[:, 128:376],
                    start=True,
                    stop=True,
                )

            def load_x(b):
                x_t = xpool.tile([C_IN, H * W], bf16, tag="x")
                if b == 0:
                    # Interleave img0's pieces across the two idle rings so
                    # each chunk's rows land as early as possible.
                    pieces = [(0, 320), (320, 640), (640, 1344), (1344, 2048),
                              (2048, 3072), (3072, 4096)]
                    for k, (lo, hi) in enumerate(pieces):
                        eng = nc.gpsimd if k % 2 == 0 else nc.sync
                        eng.dma_start(x_t[:, lo:hi], x_d[b, :, lo:hi])
                else:
                    nc.gpsimd.dma_start(x_t[:, 0:2048], x_d[b, :, 0:2048])
                    nc.gpsimd.dma_start(x_t[:, 2048:4096], x_d[b, :, 2048:4096])
                return x_t

            def moving_ap(x_t, shift, r):
                a = x_t[:, shift : shift + 1]
                return AP(
                    a.tensor, a.offset,
                    [[int(a.ap[0][0]), 128], [W, r], [1, W_OUT]],
                )

            for b in range(B_LOC):
                x_t = load_x(b)
                chunks = [
                    (c * ROWS_PER_CHUNK, min(ROWS_PER_CHUNK, H_OUT - c * ROWS_PER_CHUNK))
                    for c in range(N_CHUNKS)
                ]
                if b == B_LOC - 1:
                    # Split the final chunk so the last output DMA overlaps
                    # the tail of the matmul stream.
                    i_last, r_last = chunks.pop()
                    chunks += [(i_last, r_last - 3), (i_last + r_last - 3, 3)]
                for i0, r in chunks:
                    for half in range(N_HALF):
                        ps = ppool.tile([128, ROWS_PER_CHUNK, W_OUT], f32, tag="ps")
                        for uv in range(KH * KW):
                            u, v = divmod(uv, KW)
                            shift = (i0 + u) * W + v
                            nc.tensor.matmul(
                                ps[:, 0:r, :],
                                w_t[:, half, uv, :],
                                moving_ap(x_t, shift, r),
                                start=(uv == 0),
                                stop=(uv == KH * KW - 1),
                            )
                        o_t = opool.tile([128, ROWS_PER_CHUNK, W_OUT], f32, tag="o")
                        if half == 0:
                            nc.vector.tensor_scalar_add(
                                o_t[:, 0:r, :], ps[:, 0:r, :], b_t[:, 0:1]
                            )
                        else:
                            nc.scalar.activation(
                                o_t[:, 0:r, :],
                                ps[:, 0:r, :],
                                mybir.ActivationFunctionType.Identity,
                                bias=b_t[:, 1:2],
                            )
                        nc.sync.dma_start(
                            y_d[b, half, :, i0 : i0 + r, :], o_t[:, 0:r, :]
                        )

    nc.compile()
    if not nc.is_finalized():
        nc.finalize()
    return nc


def kernel(inputs, weights, bias, profile=False, trace_kwargs=None):
    x_bf = np.ascontiguousarray(
        np.asarray(inputs, dtype=np.float32).reshape(B_FULL, C_IN, H * W)
    ).astype(ml_dtypes.bfloat16)
    # [O, C, KH, KW] -> [C, half, KH*KW, o_local]  (lhsT layout)
    w_bf = np.ascontiguousarray(
        np.asarray(weights, dtype=np.float32)
        .reshape(N_HALF, 128, C_IN, KH * KW)
        .transpose(2, 0, 3, 1)
    ).astype(ml_dtypes.bfloat16)
    b_t = np.ascontiguousarray(
        np.asarray(bias, dtype=np.float32).reshape(N_HALF, 128).T
    )

    if "nc" not in _cached:
        _cached["nc"] = _build_nc()
    nc = _cached["nc"]

    in_maps = [
        {
            "x": x_bf[i * B_LOC : (i + 1) * B_LOC],
            "w": w_bf,
            "bias_in": b_t,
        }
        for i in range(N_CORES)
    ]
    res = run_bass_kernel_spmd(
        nc,
        in_maps,
        list(range(N_CORES)),
        trace=profile,
        **(trace_kwargs or {}),
    )
    _cached["last_result"] = res

    shards = []
    for i in range(N_CORES):
        y = res.results[i]["y"]  # [B_LOC, 2, 128, 62, 62]
        shards.append(y.reshape(B_LOC, C_OUT, H_OUT, W_OUT))
    return np.ascontiguousarray(np.concatenate(shards, axis=0), dtype=np.float32)
